# revision 27
# baseline (speedup 1.0000x reference)
"""Multi-head self-attention (B=2, T=2048, D=1024, H=16) on 8 TRN2 NeuronCores.

Sharding: batch x head-group. Core c handles batch b = c//4 and heads
h0 = 4*(c%4) .. h0+4 (Megatron-style column split of W_qkv, row split of
W_proj). Each core computes qkv projection for its heads, causal
flash-style attention for its 4 heads, and a partial output projection;
the host sums the 4 partial projections per batch (the Megatron
all-reduce realized as the unshard step) and adds b_proj.

Device algorithm (per core; matmuls in float32r = 1 cyc/row at N>=256):
  - qk^T[j, t] = sum_d W[d, j] x[t, d]  (j on partitions -> Q^T, K^T)
  - v[t, j]    = sum_d x[t, d] Wv[d, j] (t on partitions), packed into
    per-head [V_h|ones] / [ones|V_h] stationaries (parity-flipped so ctx
    lands on the lanes the ctxn head-pair packing needs)
  - S^T[kv, q] = K^T_tile.T @ Q^T, two heads row-packed per 2-bank PSUM
    pair; causal sub-diagonal tiles are skipped entirely, diagonal blocks
    masked by one strided DVE add of an inline tril constant
  - P^T = exp(S^T): one fused 2-head ScalarE activation per kv-tile
    (softmax scale is pre-folded into W_q/b_q on the host)
  - one [ctx|den] matmul per (head, kv-tile) accumulates context and the
    softmax denominators together (denominator rows come from the ones
    half of the stationary, so they cost no extra PE cycles)
  - ctxn^T = ctx * reciprocal(den): reciprocal_approx_fast must run at
    partition base 0 (HW bug at base 64), so the recip rows are
    lane-shifted to the ctx lanes with a small SBUF->SBUF DMA
  - out_partial[t, e] = sum_f ctxn^T[f, t] Wp[f, e], written as bf16
    partials (halves the output-DMA tail; host sums partials in fp32)

The attention phase is software-pipelined (stage B / normalize emitted
DEPTH-1 kv-tiles behind stage A) so the TensorE queue streams S^T matmuls
while VectorE/ScalarE run the mask+exp of earlier tiles.
"""

import sys

if "/opt/trn_rl_repo" not in sys.path:
    sys.path.insert(0, "/opt/trn_rl_repo")

from contextlib import ExitStack

import numpy as np

import concourse.bass as bass
import concourse.bacc as bacc
import concourse.tile as tile
from concourse import mybir
from concourse.bass_utils import run_bass_kernel_spmd

B, T, D, H, DH = 2, 2048, 1024, 16, 64
NCORES = 8
HL = 4          # heads per core
P = 128         # SBUF/PSUM partitions
QT = 512        # q tile (moving free dim / PSUM bank)
KT = 128        # kv tile (PSUM partition dim)
F32 = mybir.dt.float32
F32R = mybir.dt.float32r


def _build_nc() -> bass.Bass:
    nc = bacc.Bacc(None)
    Exp = mybir.ActivationFunctionType.Exp
    Ident = mybir.ActivationFunctionType.Identity

    xT_d = nc.dram_tensor("xT", [D, T], F32R, kind="ExternalInput")
    wqk_d = nc.dram_tensor("wqk", [D, 512], F32R, kind="ExternalInput")
    wv_d = nc.dram_tensor("wv", [D, 256], F32R, kind="ExternalInput")
    bqk_d = nc.dram_tensor("bqk", [512], F32, kind="ExternalInput")
    bv_d = nc.dram_tensor("bv", [256], F32, kind="ExternalInput")
    wp_d = nc.dram_tensor("wp", [256, D], F32R, kind="ExternalInput")
    out_d = nc.dram_tensor("out", [T, D], mybir.dt.bfloat16, kind="ExternalOutput")

    kv = np.arange(KT)
    tril_np = np.where(kv[:, None] <= kv[None, :], 0.0, -1e30).astype(np.float32)
    trilT_d = nc.inline_tensor(np.ascontiguousarray(tril_np.T), name="trilT")
    ident_d = nc.inline_tensor(np.eye(KT, dtype=np.float32), name="ident")

    with tile.TileContext(nc) as tc, ExitStack() as ctx:
        perm = ctx.enter_context(tc.tile_pool(name="perm", bufs=1))
        xpool = ctx.enter_context(tc.tile_pool(name="xpool", bufs=3))
        ppool = ctx.enter_context(tc.tile_pool(name="ppool", bufs=4))
        rpool = ctx.enter_context(tc.tile_pool(name="rpool", bufs=2))
        opool = ctx.enter_context(tc.tile_pool(name="opool", bufs=3))
        psA = ctx.enter_context(tc.tile_pool(name="psA", bufs=3, space="PSUM"))
        psC = ctx.enter_context(tc.tile_pool(name="psC", bufs=2, space="PSUM"))

        # --- constants (weight loads split by d-tile so matmuls start early) ---
        wqk_sb = perm.tile([P, 8, 512], F32R)
        wqk_r = wqk_d[...].rearrange("(dt p) j -> p dt j", p=P)
        nc.sync.dma_start(out=wqk_sb[:, 0:4], in_=wqk_r[:, 0:4])
        nc.sync.dma_start(out=wqk_sb[:, 4:8], in_=wqk_r[:, 4:8])
        wv_sb = perm.tile([P, 8, 256], F32R)
        nc.sync.dma_start(out=wv_sb, in_=wv_d[...].rearrange("(dt p) j -> p dt j", p=P))
        wp_sb = perm.tile([P, 2, D], F32R)
        nc.sync.dma_start(out=wp_sb, in_=wp_d[...].rearrange("(ft p) e -> p ft e", p=P))
        bqk_sb = perm.tile([P, 4], F32)
        nc.sync.dma_start(out=bqk_sb, in_=bqk_d[...].rearrange("(jt p) -> p jt", p=P))
        bv_sb = perm.tile([P, 256], F32)
        nc.gpsimd.dma_start(
            out=bv_sb, in_=bass.AP(tensor=bv_d, offset=0, ap=[[0, P], [1, 256]])
        )
        trilT_sb = perm.tile([P, KT], F32R)
        nc.sync.dma_start(out=trilT_sb, in_=trilT_d[...].bitcast(F32R))
        ident_sb = perm.tile([P, KT], F32R)
        nc.sync.dma_start(out=ident_sb, in_=ident_d[...].bitcast(F32R))

        qk_sb = perm.tile([P, 4, T], F32R)        # jt 0,1: Q^T; jt 2,3: K^T
        vaug_sb = perm.tile([P, 16, HL, 128], F32R)  # [kv, tt, h, V(64)|ones(64)]
        vaug_hh = vaug_sb.rearrange("p t (hp hh) c -> p t hp hh c", hh=2)
        ctxn_sb = perm.tile([P, 2, T], F32R)      # normalized ctx^T

        # --- phase 1: qkv production, t-quarter at a time ---
        for tt4 in range(T // QT):
            xq = xpool.tile([P, 8, QT], F32R, name=f"xq{tt4}", tag="xq")
            xq_r = xT_d[:, tt4 * QT:(tt4 + 1) * QT].rearrange("(dt p) t -> p dt t", p=P)
            nc.scalar.dma_start(out=xq[:, 0:4], in_=xq_r[:, 0:4])
            nc.scalar.dma_start(out=xq[:, 4:8], in_=xq_r[:, 4:8])
            for jt in range(4):
                ps = psA.tile([P, QT], F32, name=f"qkps{tt4}_{jt}", tag="acc")
                for dt in range(8):
                    nc.tensor.matmul(
                        out=ps,
                        lhsT=wqk_sb[:, dt, jt * 128:(jt + 1) * 128],
                        rhs=xq[:, dt, :],
                        start=(dt == 0),
                        stop=(dt == 7),
                    )
                nc.scalar.activation(
                    out=qk_sb[:, jt, tt4 * QT:(tt4 + 1) * QT],
                    in_=ps,
                    func=Ident,
                    bias=bqk_sb[:, jt:jt + 1],
                    scale=1.0,
                )
            for k in range(4):
                tt = tt4 * 4 + k
                psv = psA.tile([P, 256], F32, name=f"vps{tt}", tag="acc")
                for dt in range(8):
                    nc.tensor.matmul(
                        out=psv,
                        lhsT=xq[:, dt, k * 128:(k + 1) * 128],
                        rhs=wv_sb[:, dt, :],
                        start=(dt == 0),
                        stop=(dt == 7),
                    )
                # even heads fill [V|ones], odd heads fill [ones|V] (ones are
                # pre-memset); the flip keeps ctx rows lane-aligned with the
                # ctxn_sb head-pair packing under the fp32r dst-partition-0 rule
                vview = vaug_hh[:, tt]
                pview = psv.rearrange("p (hp hh d) -> p hp hh d", hp=2, hh=2)
                bview = bv_sb.rearrange("p (hp hh d) -> p hp hh d", hp=2, hh=2)
                nc.vector.tensor_add(
                    out=vview[:, :, 0, 0:64], in0=pview[:, :, 0, :], in1=bview[:, :, 0, :]
                )
                nc.vector.tensor_add(
                    out=vview[:, :, 1, 64:128], in0=pview[:, :, 1, :], in1=bview[:, :, 1, :]
                )
                # ones halves (memset can't write f32r): out = psv*0 + 1
                nc.vector.tensor_scalar(
                    out=vview[:, :, 0, 64:128], in0=pview[:, :, 0, :],
                    scalar1=0.0, scalar2=1.0,
                    op0=mybir.AluOpType.mult, op1=mybir.AluOpType.add,
                )
                nc.vector.tensor_scalar(
                    out=vview[:, :, 1, 0:64], in0=pview[:, :, 1, :],
                    scalar1=0.0, scalar2=1.0,
                    op0=mybir.AluOpType.mult, op1=mybir.AluOpType.add,
                )

        # --- phase 2: causal attention per head pair, software-pipelined ---
        # Stage A(j): both heads' S^T matmuls into one 2-bank psum pair, causal
        # mask adds, one fused 2-head exp. Stage B(j): both heads' [ctx|den]
        # matmuls. B(j) is emitted after A(j+DEPTH-1) so the PE queue keeps
        # streaming MM1s while VectorE/ScalarE work on the mask+exp of earlier
        # tiles (PE executes its queue in order; without the pipeline it
        # stalls on every exp and HAM re-throttles the clock).
        DEPTH = 3
        Th_by_blk = {}

        def stage_a(hp, qi, j):
            q0 = qi * QT
            qlo = max(q0, KT * j)
            qoff = qlo - q0
            s = psA.tile([P, 2 * QT], F32, name=f"s{hp}_{qi}_{j}", tag="acc")
            diag = j >= 4 * qi
            for hh in range(2):
                nc.tensor.matmul(
                    out=s[:, hh * QT + qoff: hh * QT + QT],
                    lhsT=qk_sb[hh * 64:(hh + 1) * 64, 2 + hp, j * KT:(j + 1) * KT],
                    rhs=qk_sb[hh * 64:(hh + 1) * 64, hp, qlo:q0 + QT],
                    start=True,
                    stop=not diag,
                )
                if diag:
                    # accumulate the causal -inf mask on the PE itself:
                    # out[kv,q] += trilT[q,kv] via lhsT=trilT, rhs=I. Keeps the
                    # MM1->exp chain on one engine (no DVE hop before exp).
                    nc.tensor.matmul(
                        out=s[:, hh * QT + qoff: hh * QT + qoff + KT],
                        lhsT=trilT_sb,
                        rhs=ident_sb,
                        start=False,
                        stop=True,
                    )
            p_t = ppool.tile([P, 2 * QT], F32R, name=f"p{hp}_{qi}_{j}", tag="p")
            sv = s.rearrange("p (hh c) -> p hh c", hh=2)
            pv = p_t.rearrange("p (hh c) -> p hh c", hh=2)
            nc.scalar.activation(out=pv[:, :, qoff:QT], in_=sv[:, :, qoff:QT], func=Exp)
            return p_t

        def stage_b(hp, qi, j, p_t):
            q0 = qi * QT
            njt = 4 * qi + 4
            qoff = max(q0, KT * j) - q0
            if j == 0:
                Th_by_blk[(hp, qi)] = [
                    psC.tile([P, QT], F32, name=f"T{hp}_{qi}_{hh}", tag="C")
                    for hh in range(2)
                ]
            Th = Th_by_blk[(hp, qi)]
            for hh in range(2):
                nc.tensor.matmul(
                    out=Th[hh][:, qoff:QT],
                    lhsT=vaug_sb[:, j, hp * 2 + hh, :],
                    rhs=p_t[:, hh * QT + qoff: hh * QT + QT],
                    start=(j == 0),
                    stop=(j == njt - 1),
                )
            if j == njt - 1:
                normalize(hp, qi)

        def normalize(hp, qi):
            q0 = qi * QT
            Th = Th_by_blk.pop((hp, qi))
            for hh in range(2):
                cl = hh * 64          # ctx lanes base
                rec = rpool.tile([P, QT], F32, name=f"rec{hp}_{qi}_{hh}", tag="rec")
                # reciprocal_approx_fast mis-executes at partition base 64
                # (HW-verified), so always run it at base 0.
                if hh == 1:
                    nc.vector.reciprocal_approx_fast(out=rec[0:64, :], in_=Th[hh][0:64, :])
                    nc.sync.dma_start(out=rec[64:128, :], in_=rec[0:64, :])
                else:
                    nc.vector.tensor_copy(out=rec[64:128, :], in_=Th[hh][64:128, :])
                    nc.sync.dma_start(out=rec[0:64, :], in_=rec[64:128, :])
                    nc.vector.reciprocal_approx_fast(out=rec[0:64, :], in_=rec[0:64, :])
                nc.vector.tensor_mul(
                    out=ctxn_sb[cl:cl + 64, hp, q0:q0 + QT],
                    in0=Th[hh][cl:cl + 64, :],
                    in1=rec[cl:cl + 64, :],
                )

        def proj_block(tt):
            ob = opool.tile([P, D], mybir.dt.bfloat16, name=f"ob{tt}", tag="ob")
            for et in range(2):
                ps = psA.tile([P, QT], F32, name=f"ops{tt}_{et}", tag="acc")
                for ft in range(2):
                    nc.tensor.matmul(
                        out=ps,
                        lhsT=ctxn_sb[:, ft, tt * KT:(tt + 1) * KT],
                        rhs=wp_sb[:, ft, et * QT:(et + 1) * QT],
                        start=(ft == 0),
                        stop=(ft == 1),
                    )
                nc.vector.tensor_copy(out=ob[:, et * QT:(et + 1) * QT], in_=ps)
            nc.sync.dma_start(out=out_d[tt * KT:(tt + 1) * KT, :], in_=ob)

        # --- phase 2 as one continuous global pipeline (no drain at block
        # boundaries), then phase 3 ---
        from collections import deque
        for hp in range(2):
            for qi in range(T // QT):
                pend = deque()
                for j in range(4 * qi + 4):
                    p_t = stage_a(hp, qi, j)
                    pend.append((hp, qi, j, p_t))
                    if len(pend) >= DEPTH:
                        stage_b(*pend.popleft())
                while pend:
                    stage_b(*pend.popleft())
        for tt in range(T // KT):
            proj_block(tt)

    nc.finalize()
    return nc


_NC_CACHE: list = []


def _get_nc() -> bass.Bass:
    if not _NC_CACHE:
        _NC_CACHE.append(_build_nc())
    return _NC_CACHE[0]


def _shard_inputs(x, W_qkv, b_qkv, W_proj):
    scale = np.float32(1.0 / np.sqrt(DH))
    in_maps = []
    xTs = [np.ascontiguousarray(x[b].T) for b in range(B)]
    for c in range(NCORES):
        b = c // 4
        h0 = (c % 4) * HL
        lo = h0 * DH
        wqk = np.concatenate(
            [W_qkv[:, lo:lo + 256] * scale, W_qkv[:, D + lo:D + lo + 256]], axis=1
        )
        bqk = np.concatenate([b_qkv[lo:lo + 256] * scale, b_qkv[D + lo:D + lo + 256]])
        in_maps.append({
            "xT": xTs[b],
            "wqk": np.ascontiguousarray(wqk, np.float32),
            "wv": np.ascontiguousarray(W_qkv[:, 2 * D + lo:2 * D + lo + 256], np.float32),
            "bqk": np.ascontiguousarray(bqk, np.float32),
            "bv": np.ascontiguousarray(b_qkv[2 * D + lo:2 * D + lo + 256], np.float32),
            "wp": np.ascontiguousarray(W_proj[lo:lo + 256, :], np.float32),
        })
    return in_maps


def kernel(x, W_qkv, b_qkv, W_proj, b_proj, _trace=False, _tmpdir=None):
    x = np.asarray(x, np.float32)
    W_qkv = np.asarray(W_qkv, np.float32)
    b_qkv = np.asarray(b_qkv, np.float32)
    W_proj = np.asarray(W_proj, np.float32)
    b_proj = np.asarray(b_proj, np.float32)

    nc = _get_nc()
    in_maps = _shard_inputs(x, W_qkv, b_qkv, W_proj)
    kw = {}
    if _trace:
        kw = dict(trace=True, tmpdir=_tmpdir)
    r = run_bass_kernel_spmd(nc, in_maps, core_ids=list(range(NCORES)), **kw)
    out = np.zeros((B, T, D), np.float32)
    for c in range(NCORES):
        out[c // 4] += np.asarray(r.results[c]["out"], np.float32)
    out += b_proj[None, None, :]
    if _trace:
        return out, r
    return out


# revision 29
# speedup vs baseline: 1.1605x; 1.1605x over previous
"""Multi-head self-attention (B=2, T=2048, D=1024, H=16) on 8 TRN2 NeuronCores.

Sharding: batch x head-group. Core c handles batch b = c//4 and heads
h0 = 4*(c%4) .. h0+4 (Megatron-style column split of W_qkv, row split of
W_proj). Each core computes qkv projection for its heads, causal
flash-style attention for its 4 heads, and a partial output projection;
the host sums the 4 partial projections per batch (the Megatron
all-reduce realized as the unshard step) and adds b_proj.

Device algorithm (per core; matmuls in float32r = 1 cyc/row at N>=256):
  - qk^T[j, t] = sum_d W[d, j] x[t, d]  (j on partitions -> Q^T, K^T)
  - v[t, j]    = sum_d x[t, d] Wv[d, j] (t on partitions), packed into
    per-head [V_h|ones] / [ones|V_h] stationaries (parity-flipped so ctx
    lands on the lanes the ctxn head-pair packing needs)
  - S^T[kv, q] = K^T_tile.T @ Q^T, two heads row-packed per 2-bank PSUM
    pair; causal sub-diagonal tiles are skipped entirely, diagonal blocks
    masked by one strided DVE add of an inline tril constant
  - P^T = exp(S^T): one fused 2-head ScalarE activation per kv-tile
    (softmax scale is pre-folded into W_q/b_q on the host)
  - one [ctx|den] matmul per (head, kv-tile) accumulates context and the
    softmax denominators together (denominator rows come from the ones
    half of the stationary, so they cost no extra PE cycles)
  - ctxn^T = ctx * reciprocal(den): reciprocal_approx_fast must run at
    partition base 0 (HW bug at base 64), so the recip rows are
    lane-shifted to the ctx lanes with a small SBUF->SBUF DMA
  - out_partial[t, e] = sum_f ctxn^T[f, t] Wp[f, e], written as bf16
    partials (halves the output-DMA tail; host sums partials in fp32)

The attention phase is software-pipelined (stage B / normalize emitted
DEPTH-1 kv-tiles behind stage A) so the TensorE queue streams S^T matmuls
while VectorE/ScalarE run the mask+exp of earlier tiles.
"""

import sys

if "/opt/trn_rl_repo" not in sys.path:
    sys.path.insert(0, "/opt/trn_rl_repo")

from contextlib import ExitStack

import numpy as np

import concourse.bass as bass
import concourse.bacc as bacc
import concourse.tile as tile
from concourse import mybir
from concourse.bass_utils import run_bass_kernel_spmd

B, T, D, H, DH = 2, 2048, 1024, 16, 64
NCORES = 8
HL = 4          # heads per core
P = 128         # SBUF/PSUM partitions
QT = 512        # q tile (moving free dim / PSUM bank)
KT = 128        # kv tile (PSUM partition dim)
F32 = mybir.dt.float32
F32R = mybir.dt.float32r


def _build_nc() -> bass.Bass:
    nc = bacc.Bacc(None)
    Exp = mybir.ActivationFunctionType.Exp
    Ident = mybir.ActivationFunctionType.Identity

    xT_d = nc.dram_tensor("xT", [D, T], F32R, kind="ExternalInput")
    wqk_d = nc.dram_tensor("wqk", [D, 512], F32R, kind="ExternalInput")
    wv_d = nc.dram_tensor("wv", [D, 256], F32R, kind="ExternalInput")
    bqk_d = nc.dram_tensor("bqk", [512], F32, kind="ExternalInput")
    bv_d = nc.dram_tensor("bv", [256], F32, kind="ExternalInput")
    wp_d = nc.dram_tensor("wp", [256, D], F32R, kind="ExternalInput")
    out_d = nc.dram_tensor("out", [T, D], mybir.dt.bfloat16, kind="ExternalOutput")

    kv = np.arange(KT)
    tril_np = np.where(kv[:, None] <= kv[None, :], 0.0, -1e30).astype(np.float32)
    tril_d = nc.inline_tensor(tril_np, name="tril")
    maskw_np = np.concatenate([np.full((KT, KT), -1e30, np.float32), tril_np], axis=1)
    maskw_d = nc.inline_tensor(maskw_np, name="maskw")

    with tile.TileContext(nc) as tc, ExitStack() as ctx:
        perm = ctx.enter_context(tc.tile_pool(name="perm", bufs=1))
        xpool = ctx.enter_context(tc.tile_pool(name="xpool", bufs=2))
        ppool = ctx.enter_context(tc.tile_pool(name="ppool", bufs=6))
        rpool = ctx.enter_context(tc.tile_pool(name="rpool", bufs=3))
        opool = ctx.enter_context(tc.tile_pool(name="opool", bufs=3))
        psA = ctx.enter_context(tc.tile_pool(name="psA", bufs=3, space="PSUM"))
        psC = ctx.enter_context(tc.tile_pool(name="psC", bufs=2, space="PSUM"))

        # --- constants (weight loads split by d-tile so matmuls start early) ---
        wqk_sb = perm.tile([P, 8, 512], F32R)
        nc.sync.dma_start(out=wqk_sb, in_=wqk_d[...].rearrange("(dt p) j -> p dt j", p=P))
        wv_sb = perm.tile([P, 8, 256], F32R)
        nc.sync.dma_start(out=wv_sb, in_=wv_d[...].rearrange("(dt p) j -> p dt j", p=P))
        wp_sb = perm.tile([P, 2, D], F32R)
        nc.sync.dma_start(out=wp_sb, in_=wp_d[...].rearrange("(ft p) e -> p ft e", p=P))
        bqk_sb = perm.tile([P, 4], F32)
        nc.sync.dma_start(out=bqk_sb, in_=bqk_d[...].rearrange("(jt p) -> p jt", p=P))
        bv_sb = perm.tile([P, 256], F32)
        nc.gpsimd.dma_start(
            out=bv_sb, in_=bass.AP(tensor=bv_d, offset=0, ap=[[0, P], [1, 256]])
        )
        tril_sb = perm.tile([P, KT], F32)
        nc.sync.dma_start(out=tril_sb, in_=tril_d[...])
        maskw_sb = perm.tile([P, 2 * KT], F32)
        nc.sync.dma_start(out=maskw_sb, in_=maskw_d[...])

        qk_sb = perm.tile([P, 4, T], F32R)        # jt 0,1: Q^T; jt 2,3: K^T
        vaug_sb = perm.tile([P, 16, HL, 128], F32R)  # [kv, tt, h, V(64)|ones(64)]
        vaug_hh = vaug_sb.rearrange("p t (hp hh) c -> p t hp hh c", hh=2)
        ctxn_sb = perm.tile([P, 2, T], F32R)      # normalized ctx^T

        # --- phase 1: qkv production, t-quarter at a time ---
        for tt4 in range(T // QT):
            xq = xpool.tile([P, 8, QT], F32R, name=f"xq{tt4}", tag="xq")
            nc.scalar.dma_start(
                out=xq,
                in_=xT_d[:, tt4 * QT:(tt4 + 1) * QT].rearrange("(dt p) t -> p dt t", p=P),
            )
            for jt in range(4):
                ps = psA.tile([P, QT], F32, name=f"qkps{tt4}_{jt}", tag="acc")
                for dt in range(8):
                    nc.tensor.matmul(
                        out=ps,
                        lhsT=wqk_sb[:, dt, jt * 128:(jt + 1) * 128],
                        rhs=xq[:, dt, :],
                        start=(dt == 0),
                        stop=(dt == 7),
                    )
                nc.scalar.activation(
                    out=qk_sb[:, jt, tt4 * QT:(tt4 + 1) * QT],
                    in_=ps,
                    func=Ident,
                    bias=bqk_sb[:, jt:jt + 1],
                    scale=1.0,
                )
            for k in range(4):
                tt = tt4 * 4 + k
                psv = psA.tile([P, 256], F32, name=f"vps{tt}", tag="acc")
                for dt in range(8):
                    nc.tensor.matmul(
                        out=psv,
                        lhsT=xq[:, dt, k * 128:(k + 1) * 128],
                        rhs=wv_sb[:, dt, :],
                        start=(dt == 0),
                        stop=(dt == 7),
                    )
                # even heads fill [V|ones], odd heads fill [ones|V] (ones are
                # pre-memset); the flip keeps ctx rows lane-aligned with the
                # ctxn_sb head-pair packing under the fp32r dst-partition-0 rule
                vview = vaug_hh[:, tt]
                pview = psv.rearrange("p (hp hh d) -> p hp hh d", hp=2, hh=2)
                bview = bv_sb.rearrange("p (hp hh d) -> p hp hh d", hp=2, hh=2)
                nc.vector.tensor_add(
                    out=vview[:, :, 0, 0:64], in0=pview[:, :, 0, :], in1=bview[:, :, 0, :]
                )
                nc.vector.tensor_add(
                    out=vview[:, :, 1, 64:128], in0=pview[:, :, 1, :], in1=bview[:, :, 1, :]
                )
                # ones halves (memset can't write f32r): out = psv*0 + 1
                nc.vector.tensor_scalar(
                    out=vview[:, :, 0, 64:128], in0=pview[:, :, 0, :],
                    scalar1=0.0, scalar2=1.0,
                    op0=mybir.AluOpType.mult, op1=mybir.AluOpType.add,
                )
                nc.vector.tensor_scalar(
                    out=vview[:, :, 1, 0:64], in0=pview[:, :, 1, :],
                    scalar1=0.0, scalar2=1.0,
                    op0=mybir.AluOpType.mult, op1=mybir.AluOpType.add,
                )

        # --- phase 2: causal attention per head pair, software-pipelined ---
        # Stage A(j): both heads' S^T matmuls into one 2-bank psum pair, causal
        # mask adds, one fused 2-head exp. Stage B(j): both heads' [ctx|den]
        # matmuls. B(j) is emitted after A(j+DEPTH-1) so the PE queue keeps
        # streaming MM1s while VectorE/ScalarE work on the mask+exp of earlier
        # tiles (PE executes its queue in order; without the pipeline it
        # stalls on every exp and HAM re-throttles the clock).
        DEPTH = 3
        Th_by_blk = {}

        def tile_geom(qi, j):
            # fp32r matmuls under N=256 run at 1/4 rate; widen the N=128
            # diagonal tail to N=256 (extra columns are fully masked ->
            # exp()==0 -> the wider accumulations are exact no-ops)
            q0 = qi * QT
            qlo = max(q0, KT * j)
            if qlo - q0 == 384:
                qlo = q0 + 256
            return qlo - q0

        def stage_a(hp, qi, j):
            q0 = qi * QT
            qoff = tile_geom(qi, j)
            qlo = q0 + qoff
            s = psA.tile([P, 2 * QT], F32, name=f"s{hp}_{qi}_{j}", tag="acc")
            for hh in range(2):
                nc.tensor.matmul(
                    out=s[:, hh * QT + qoff: hh * QT + QT],
                    lhsT=qk_sb[hh * 64:(hh + 1) * 64, 2 + hp, j * KT:(j + 1) * KT],
                    rhs=qk_sb[hh * 64:(hh + 1) * 64, hp, qlo:q0 + QT],
                    start=True,
                    stop=True,
                )
            if j >= 4 * qi:
                pre = KT * j - qlo      # 128 for the widened tail, else 0
                msrc = maskw_sb if pre else tril_sb
                w = pre + KT
                sm = s.rearrange("p (hh c) -> p hh c", hh=2)[:, :, qoff:qoff + w]
                mask_b = bass.AP(
                    tensor=msrc.tensor,
                    offset=msrc.offset,
                    ap=[msrc.ap[0], [0, 2], [1, w]],
                )
                nc.vector.tensor_add(out=sm, in0=sm, in1=mask_b)
            p_t = ppool.tile([P, 2 * QT], F32R, name=f"p{hp}_{qi}_{j}", tag="p")
            sv = s.rearrange("p (hh c) -> p hh c", hh=2)
            pv = p_t.rearrange("p (hh c) -> p hh c", hh=2)
            nc.scalar.activation(out=pv[:, :, qoff:QT], in_=sv[:, :, qoff:QT], func=Exp)
            return p_t

        def stage_b(hp, qi, j, p_t):
            q0 = qi * QT
            njt = 4 * qi + 4
            qoff = tile_geom(qi, j)
            if j == 0:
                Th_by_blk[(hp, qi)] = [
                    psC.tile([P, QT], F32, name=f"T{hp}_{qi}_{hh}", tag="C")
                    for hh in range(2)
                ]
            Th = Th_by_blk[(hp, qi)]
            for hh in range(2):
                nc.tensor.matmul(
                    out=Th[hh][:, qoff:QT],
                    lhsT=vaug_sb[:, j, hp * 2 + hh, :],
                    rhs=p_t[:, hh * QT + qoff: hh * QT + QT],
                    start=(j == 0),
                    stop=(j == njt - 1),
                )
            if j == njt - 1:
                normalize(hp, qi)

        def normalize(hp, qi):
            q0 = qi * QT
            Th = Th_by_blk.pop((hp, qi))
            for hh in range(2):
                cl = hh * 64          # ctx lanes base
                rec = rpool.tile([P, QT], F32, name=f"rec{hp}_{qi}_{hh}", tag="rec")
                # reciprocal_approx_fast mis-executes at partition base 64
                # (HW-verified), so always run it at base 0.
                if hh == 1:
                    nc.vector.reciprocal_approx_fast(out=rec[0:64, :], in_=Th[hh][0:64, :])
                    nc.sync.dma_start(out=rec[64:128, :], in_=rec[0:64, :])
                else:
                    nc.vector.tensor_copy(out=rec[64:128, :], in_=Th[hh][64:128, :])
                    nc.sync.dma_start(out=rec[0:64, :], in_=rec[64:128, :])
                    nc.vector.reciprocal_approx_fast(out=rec[0:64, :], in_=rec[0:64, :])
                nc.vector.tensor_mul(
                    out=ctxn_sb[cl:cl + 64, hp, q0:q0 + QT],
                    in0=Th[hh][cl:cl + 64, :],
                    in1=rec[cl:cl + 64, :],
                )

        def proj_block(tt):
            ob = opool.tile([P, D], mybir.dt.bfloat16, name=f"ob{tt}", tag="ob")
            for et in range(2):
                ps = psA.tile([P, QT], F32, name=f"ops{tt}_{et}", tag="acc")
                for ft in range(2):
                    nc.tensor.matmul(
                        out=ps,
                        lhsT=ctxn_sb[:, ft, tt * KT:(tt + 1) * KT],
                        rhs=wp_sb[:, ft, et * QT:(et + 1) * QT],
                        start=(ft == 0),
                        stop=(ft == 1),
                    )
                nc.vector.tensor_copy(out=ob[:, et * QT:(et + 1) * QT], in_=ps)
            nc.sync.dma_start(out=out_d[tt * KT:(tt + 1) * KT, :], in_=ob)

        # --- phase 2 as one continuous global pipeline (no drain at block
        # boundaries), then phase 3 ---
        from collections import deque
        for hp in range(2):
            for qi in range(T // QT):
                pend = deque()
                for j in range(4 * qi + 4):
                    p_t = stage_a(hp, qi, j)
                    pend.append((hp, qi, j, p_t))
                    if len(pend) >= DEPTH:
                        stage_b(*pend.popleft())
                while pend:
                    stage_b(*pend.popleft())
        for tt in range(T // KT):
            proj_block(tt)

    nc.finalize()
    return nc


_NC_CACHE: list = []


def _get_nc() -> bass.Bass:
    if not _NC_CACHE:
        _NC_CACHE.append(_build_nc())
    return _NC_CACHE[0]


def _shard_inputs(x, W_qkv, b_qkv, W_proj):
    scale = np.float32(1.0 / np.sqrt(DH))
    in_maps = []
    xTs = [np.ascontiguousarray(x[b].T) for b in range(B)]
    for c in range(NCORES):
        b = c // 4
        h0 = (c % 4) * HL
        lo = h0 * DH
        wqk = np.concatenate(
            [W_qkv[:, lo:lo + 256] * scale, W_qkv[:, D + lo:D + lo + 256]], axis=1
        )
        bqk = np.concatenate([b_qkv[lo:lo + 256] * scale, b_qkv[D + lo:D + lo + 256]])
        in_maps.append({
            "xT": xTs[b],
            "wqk": np.ascontiguousarray(wqk, np.float32),
            "wv": np.ascontiguousarray(W_qkv[:, 2 * D + lo:2 * D + lo + 256], np.float32),
            "bqk": np.ascontiguousarray(bqk, np.float32),
            "bv": np.ascontiguousarray(b_qkv[2 * D + lo:2 * D + lo + 256], np.float32),
            "wp": np.ascontiguousarray(W_proj[lo:lo + 256, :], np.float32),
        })
    return in_maps


def kernel(x, W_qkv, b_qkv, W_proj, b_proj, _trace=False, _tmpdir=None):
    x = np.asarray(x, np.float32)
    W_qkv = np.asarray(W_qkv, np.float32)
    b_qkv = np.asarray(b_qkv, np.float32)
    W_proj = np.asarray(W_proj, np.float32)
    b_proj = np.asarray(b_proj, np.float32)

    nc = _get_nc()
    in_maps = _shard_inputs(x, W_qkv, b_qkv, W_proj)
    kw = {}
    if _trace:
        kw = dict(trace=True, tmpdir=_tmpdir)
    r = run_bass_kernel_spmd(nc, in_maps, core_ids=list(range(NCORES)), **kw)
    out = np.zeros((B, T, D), np.float32)
    for c in range(NCORES):
        out[c // 4] += np.asarray(r.results[c]["out"], np.float32)
    out += b_proj[None, None, :]
    if _trace:
        return out, r
    return out
